# revision 10
# baseline (speedup 1.0000x reference)
"""Multi-head attention (B=8, S=1024, D=1024, H=16) on 8 TRN2 NeuronCores.

Sharding: pure data parallel — batch element b on core b. Weights are
broadcast to every core. No collectives.

Per-core algorithm (X: [S, D] for one batch element):
  1. X^T via PE transposes (fp32 has no DMA transpose).
  2. QK^T = W_in[:, :2D]^T @ X  -> [2D, S] "transposed" projection, so
     Q^T/K^T land with head dims on partitions (what scores need).
  3. V = X @ W_in[:, 2D:]       -> [S, D] natural layout, stored as
     V_aug[sk, head, 65] with a ones column appended (col 64).
  4. Per head pair (2 heads share a 128-partition group):
     S^T[sk, sq] = K_h^T.T @ Q_h^T  (K=64 contraction; the two heads run
     concurrently on PE row-groups 0-63 / 64-127),
     exp on ScalarE with scale=1/8 (softmax max-subtraction is skipped:
     scores are ~N(0,1), exp cannot overflow),
     PV: [V_h | 1]^T @ exp -> rows 0-63 unnormalized out^T, row 64 the
     softmax denominator.
  5. Normalize: reciprocal of row 64, GpSimd partition-broadcast, one
     DVE multiply per pair -> attn_out^T (f32r).
  6. Y = attn_out @ W_out + b_out in natural layout; DMA out.

All matmul operands are float32r (full-rate fp32 matmul; ~1.5e-4 rel err
measured on HW vs fp64 for K=1024).
"""

import os
import sys

sys.path.insert(0, "/opt/trn_rl_repo")

import numpy as np

import concourse.bacc as bacc
import concourse.mybir as mybir
from concourse.bass_utils import run_bass_kernel_spmd
from concourse.masks import make_identity
from concourse.tile import TileContext

B = 8
S = 1024
D = 1024
H = 16
DK = D // H  # 64
P = 128
ST = S // P   # 8 s-tiles
DT = D // P   # 8 d-tiles
NTQK = 2 * D // P  # 16 n-tiles for the Q|K part
PAIRS = H // 2     # 8 head pairs
SC = S // 512      # 2 sq chunks of 512

f32 = mybir.dt.float32
f32r = mybir.dt.float32r
EXP = mybir.ActivationFunctionType.Exp
MULT = mybir.AluOpType.mult
ADD = mybir.AluOpType.add


def build_nc():
    nc = bacc.Bacc()
    X = nc.dram_tensor("X", [S, D], f32, kind="ExternalInput")
    W_in = nc.dram_tensor("W_in", [D, 3 * D], f32, kind="ExternalInput")
    b_in = nc.dram_tensor("b_in", [3 * D], f32, kind="ExternalInput")
    W_out = nc.dram_tensor("W_out", [D, D], f32, kind="ExternalInput")
    b_out = nc.dram_tensor("b_out", [D], f32, kind="ExternalInput")
    out = nc.dram_tensor("out", [S, D], f32, kind="ExternalOutput")

    w_in_kp = W_in.rearrange("(ko p) n -> p ko n", p=P)  # [128, 8, 3072]
    w_out_kp = W_out.rearrange("(ko p) n -> p ko n", p=P)  # [128, 8, 1024]

    with TileContext(nc) as tc:
        const = tc.alloc_tile_pool(name="const", bufs=1)
        psum = tc.alloc_tile_pool(name="psum", bufs=4, space="PSUM")
        pvps = tc.alloc_tile_pool(name="pvps", bufs=4, space="PSUM")

        identity = const.tile([P, P], f32)
        make_identity(nc, identity[:])
        # b_in for the Q|K part, striped so bias is per-partition: [128, 16]
        bqk = const.tile([P, NTQK], f32)
        nc.sync.dma_start(bqk[:], b_in[0 : 2 * D].rearrange("(o p) -> p o", p=P))
        # b_in V part + b_out broadcast across partitions (rows staged in pa)
        bv_bc = const.tile([P, D], f32)
        bout_bc = const.tile([P, D], f32)
        ones4 = const.tile([P, ST, H, 1], f32)
        nc.vector.memset(ones4[:], 1.0)
        ones64f = const.tile([1, DK], f32)
        nc.vector.memset(ones64f[:], 1.0)
        ones64 = const.tile([1, DK], f32r)
        nc.vector.tensor_copy(ones64[:], ones64f[:])

        # ---------------- resident tensors ----------------
        qkT_pool = tc.alloc_tile_pool(name="qkT", bufs=1)
        qkT = qkT_pool.tile([P, NTQK, S], f32r)  # 8 MB
        vaug_pool = tc.alloc_tile_pool(name="vaug", bufs=1)
        v_aug = vaug_pool.tile([P, ST, H, DK + 1], f32r)  # 4.26 MB
        nc.vector.tensor_copy(v_aug[:, :, :, DK : DK + 1], ones4[:])

        # ---------------- phase A: X^T ----------------
        pa = tc.alloc_tile_pool(name="phaseA", bufs=1)
        bv_row = pa.tile([1, D], f32)
        nc.sync.dma_start(bv_row[:], b_in[None, 2 * D : 3 * D])
        nc.gpsimd.partition_broadcast(bv_bc[:], bv_row[:])
        bout_row = pa.tile([1, D], f32)
        nc.sync.dma_start(bout_row[:], b_out[None, :])
        nc.gpsimd.partition_broadcast(bout_bc[:], bout_row[:])
        with tc.tile_pool(name="xstage", bufs=2) as xstage:
            xT = pa.tile([P, DT, S], f32r)  # 4 MB, lives through B+C
            for si in range(ST):
                x_tile = xstage.tile([P, D], f32, tag="x")
                nc.sync.dma_start(x_tile[:], X[si * P : (si + 1) * P, :])
                for dj in range(DT):
                    tp = psum.tile([P, P], f32, tag="mm")
                    nc.tensor.transpose(
                        tp[:], x_tile[:, dj * P : (dj + 1) * P], identity[:]
                    )
                    nc.vector.tensor_copy(xT[:, dj, si * P : (si + 1) * P], tp[:])

            # ---------------- phase B: Q^T | K^T projection ----------------
            with tc.tile_pool(name="wqk", bufs=2) as wqk:
                for nt in range(NTQK):
                    w_tile = wqk.tile([P, DT, P], f32r, tag="w")
                    nc.sync.dma_start(
                        w_tile[:],
                        w_in_kp[:, :, nt * P : (nt + 1) * P].bitcast(f32r),
                    )
                    for sc in range(SC):
                        ps = psum.tile([P, 512], f32, tag="mm")
                        for dk in range(DT):
                            nc.tensor.matmul(
                                ps[:],
                                w_tile[:, dk, :],
                                xT[:, dk, sc * 512 : (sc + 1) * 512],
                                start=(dk == 0),
                                stop=(dk == DT - 1),
                            )
                        nc.vector.tensor_scalar_add(
                            qkT[:, nt, sc * 512 : (sc + 1) * 512],
                            ps[:],
                            bqk[:, nt : nt + 1],
                        )

            # ---------------- phase C: V projection (natural) ----------------
            with tc.tile_pool(name="wv", bufs=2) as wv:
                for ncx in range(SC):
                    wv_tile = wv.tile([P, DT, 512], f32r, tag="wv")
                    nc.sync.dma_start(
                        wv_tile[:],
                        w_in_kp[
                            :, :, 2 * D + ncx * 512 : 2 * D + (ncx + 1) * 512
                        ].bitcast(f32r),
                    )
                    for st in range(ST):
                        ps = psum.tile([P, 512], f32, tag="mm")
                        for dk in range(DT):
                            nc.tensor.matmul(
                                ps[:],
                                xT[:, dk, st * P : (st + 1) * P],
                                wv_tile[:, dk, :],
                                start=(dk == 0),
                                stop=(dk == DT - 1),
                            )
                        nc.vector.tensor_tensor(
                            v_aug[:, st, ncx * 8 : (ncx + 1) * 8, 0:DK],
                            ps[:].rearrange("p (h d) -> p h d", d=DK),
                            bv_bc[:, ncx * 512 : (ncx + 1) * 512].rearrange(
                                "p (h d) -> p h d", d=DK
                            ),
                            ADD,
                        )

        pa.release()

        # ---------------- phase D: attention ----------------
        attnT_pool = tc.alloc_tile_pool(name="attnT", bufs=1)
        attnT = attnT_pool.tile([P, DT, S], f32r)  # 4 MB
        wout_pool = tc.alloc_tile_pool(name="wout", bufs=1)
        wout = wout_pool.tile([P, DT, D], f32r)  # 4 MB; prefetch during D
        nc.sync.dma_start(wout[:], w_out_kp[:].bitcast(f32r))

        with (
            tc.tile_pool(name="expp", bufs=4) as expp,
            tc.tile_pool(name="unno", bufs=2) as unno,
            tc.tile_pool(name="rrow", bufs=2) as rrowp,
        ):
            for pr in range(PAIRS):
                for sc in range(SC):
                    pv = [pvps.tile([P, 512], f32, tag="pv", name=f"pv{i}") for i in range(2)]
                    exps = {}
                    # software pipeline: scores(sk) on PE, exp(sk) on ACT,
                    # pv(sk-1) on PE
                    for sk in range(ST + 1):
                        if sk < ST:
                            for hh in range(2):
                                base = hh * DK
                                sps = psum.tile([P, 512], f32, tag="mm")
                                nc.tensor.matmul(
                                    sps[:],
                                    qkT[
                                        base : base + DK,
                                        PAIRS + pr,
                                        sk * P : (sk + 1) * P,
                                    ],
                                    qkT[
                                        base : base + DK,
                                        pr,
                                        sc * 512 : (sc + 1) * 512,
                                    ],
                                    start=True,
                                    stop=True,
                                )
                                ex = expp.tile([P, 512], f32r, tag="ex")
                                nc.scalar.activation(
                                    ex[:], sps[:], EXP, scale=1.0 / np.sqrt(DK)
                                )
                                exps[(sk, hh)] = ex
                        if sk >= 1:
                            for hh in range(2):
                                h = 2 * pr + hh
                                nc.tensor.matmul(
                                    pv[hh][0 : DK + 1, :],
                                    v_aug[:, sk - 1, h, :],
                                    exps.pop((sk - 1, hh))[:],
                                    start=(sk - 1 == 0),
                                    stop=(sk - 1 == ST - 1),
                                )
                    unnorm = unno.tile([P, 512], f32, tag="un")
                    rrow = [rrowp.tile([1, 512], f32r, tag="rr", name=f"rrow{i}")
                            for i in range(2)]
                    for hh in range(2):
                        base = hh * DK
                        nc.vector.tensor_copy(
                            unnorm[base : base + DK, :], pv[hh][0:DK, :]
                        )
                        with nc.allow_low_precision(reason="f32r recip, ~1e-4"):
                            nc.vector.reciprocal(
                                rrow[hh][:], pv[hh][DK : DK + 1, :]
                            )
                        # ones[1,64].T @ recip[1,512]: broadcast into PSUM
                        rb = psum.tile([DK, 512], f32, tag="mm", name="rb")
                        nc.tensor.matmul(
                            rb[:], ones64[:], rrow[hh][:], start=True, stop=True
                        )
                        nc.vector.tensor_tensor(
                            attnT[
                                base : base + DK, pr, sc * 512 : (sc + 1) * 512
                            ],
                            unnorm[base : base + DK, :],
                            rb[:],
                            MULT,
                        )

        # ---------------- phase E: output projection ----------------
        with tc.tile_pool(name="ypool", bufs=3) as ypool:
            for st in range(ST):
                for ncx in range(SC):
                    ps = psum.tile([P, 512], f32, tag="mm")
                    for dk in range(DT):
                        nc.tensor.matmul(
                            ps[:],
                            attnT[:, dk, st * P : (st + 1) * P],
                            wout[:, dk, ncx * 512 : (ncx + 1) * 512],
                            start=(dk == 0),
                            stop=(dk == DT - 1),
                        )
                    y = ypool.tile([P, 512], f32, tag="y")
                    nc.vector.tensor_tensor(
                        y[:], ps[:], bout_bc[:, ncx * 512 : (ncx + 1) * 512], ADD
                    )
                    nc.sync.dma_start(
                        out[st * P : (st + 1) * P, ncx * 512 : (ncx + 1) * 512],
                        y[:],
                    )

        for pool in (wout_pool, attnT_pool, vaug_pool, qkT_pool, pvps, psum, const):
            pool.release()

    nc.finalize()
    return nc


_NC_CACHE = {}


def get_nc():
    if "nc" not in _NC_CACHE:
        _NC_CACHE["nc"] = build_nc()
    return _NC_CACHE["nc"]


def kernel(X, W_in, b_in, W_out, b_out):
    X = np.ascontiguousarray(np.asarray(X, dtype=np.float32))
    W_in = np.ascontiguousarray(np.asarray(W_in, dtype=np.float32))
    b_in = np.ascontiguousarray(np.asarray(b_in, dtype=np.float32))
    W_out = np.ascontiguousarray(np.asarray(W_out, dtype=np.float32))
    b_out = np.ascontiguousarray(np.asarray(b_out, dtype=np.float32))

    nc = get_nc()
    in_maps = [
        {"X": X[i], "W_in": W_in, "b_in": b_in, "W_out": W_out, "b_out": b_out}
        for i in range(B)
    ]
    res = run_bass_kernel_spmd(nc, in_maps, core_ids=list(range(B)))
    return np.stack([res.results[i]["out"] for i in range(B)], axis=0)


# revision 11
# speedup vs baseline: 1.5842x; 1.5842x over previous
"""Multi-head attention (B=8, S=1024, D=1024, H=16) on 8 TRN2 NeuronCores.

Sharding: pure data parallel — batch element b on core b. Weights are
broadcast to every core. No collectives.

Per-core algorithm (X: [S, D] for one batch element):
  1. X^T via PE transposes (fp32 has no DMA transpose).
  2. QK^T = W_in[:, :2D]^T @ X  -> [2D, S] "transposed" projection, so
     Q^T/K^T land with head dims on partitions (what scores need).
  3. V = X @ W_in[:, 2D:]       -> [S, D] natural layout, stored as
     V_aug[sk, head, 65] with a ones column appended (col 64).
  4. Per head pair (2 heads share a 128-partition group):
     S^T[sk, sq] = K_h^T.T @ Q_h^T  (K=64 contraction; the two heads run
     concurrently on PE row-groups 0-63 / 64-127),
     exp on ScalarE with scale=1/8 (softmax max-subtraction is skipped:
     scores are ~N(0,1), exp cannot overflow),
     PV: [V_h | 1]^T @ exp -> rows 0-63 unnormalized out^T, row 64 the
     softmax denominator.
  5. Normalize: reciprocal of row 64, GpSimd partition-broadcast, one
     DVE multiply per pair -> attn_out^T (f32r).
  6. Y = attn_out @ W_out + b_out in natural layout; DMA out.

All matmul operands are float32r (full-rate fp32 matmul; ~1.5e-4 rel err
measured on HW vs fp64 for K=1024).
"""

import os
import sys

sys.path.insert(0, "/opt/trn_rl_repo")

import numpy as np

import concourse.bacc as bacc
import concourse.mybir as mybir
from concourse.bass_utils import run_bass_kernel_spmd
from concourse.masks import make_identity
from concourse.tile import TileContext

B = 8
S = 1024
D = 1024
H = 16
DK = D // H  # 64
P = 128
ST = S // P   # 8 s-tiles
DT = D // P   # 8 d-tiles
NTQK = 2 * D // P  # 16 n-tiles for the Q|K part
PAIRS = H // 2     # 8 head pairs
SC = S // 512      # 2 sq chunks of 512

f32 = mybir.dt.float32
f32r = mybir.dt.float32r
EXP = mybir.ActivationFunctionType.Exp
MULT = mybir.AluOpType.mult
ADD = mybir.AluOpType.add


def build_nc():
    nc = bacc.Bacc()
    X = nc.dram_tensor("X", [S, D], f32, kind="ExternalInput")
    W_in = nc.dram_tensor("W_in", [D, 3 * D], f32, kind="ExternalInput")
    b_in = nc.dram_tensor("b_in", [3 * D], f32, kind="ExternalInput")
    W_out = nc.dram_tensor("W_out", [D, D], f32, kind="ExternalInput")
    b_out = nc.dram_tensor("b_out", [D], f32, kind="ExternalInput")
    out = nc.dram_tensor("out", [S, D], f32, kind="ExternalOutput")

    w_in_kp = W_in.rearrange("(ko p) n -> p ko n", p=P)  # [128, 8, 3072]
    w_out_kp = W_out.rearrange("(ko p) n -> p ko n", p=P)  # [128, 8, 1024]

    with TileContext(nc) as tc:
        const = tc.alloc_tile_pool(name="const", bufs=1)
        psum = tc.alloc_tile_pool(name="psum", bufs=4, space="PSUM")
        pvps = tc.alloc_tile_pool(name="pvps", bufs=4, space="PSUM")

        identity = const.tile([P, P], f32)
        make_identity(nc, identity[:])
        # b_in for the Q|K part, striped so bias is per-partition: [128, 16]
        bqk = const.tile([P, NTQK], f32)
        nc.sync.dma_start(bqk[:], b_in[0 : 2 * D].rearrange("(o p) -> p o", p=P))
        # b_in V part + b_out broadcast across partitions (rows staged in pa)
        bv_bc = const.tile([P, D], f32)
        bout_bc = const.tile([P, D], f32)
        ones4 = const.tile([P, ST, H, 1], f32)
        nc.vector.memset(ones4[:], 1.0)

        # ---------------- resident tensors ----------------
        qkT_pool = tc.alloc_tile_pool(name="qkT", bufs=1)
        qkT = qkT_pool.tile([P, NTQK, S], f32r)  # 8 MB
        vaug_pool = tc.alloc_tile_pool(name="vaug", bufs=1)
        v_aug = vaug_pool.tile([P, ST, H, DK + 1], f32r)  # 4.26 MB
        nc.vector.tensor_copy(v_aug[:, :, :, DK : DK + 1], ones4[:])

        # ---------------- phase A: X^T ----------------
        pa = tc.alloc_tile_pool(name="phaseA", bufs=1)
        bv_row = pa.tile([1, D], f32)
        nc.sync.dma_start(bv_row[:], b_in[None, 2 * D : 3 * D])
        nc.gpsimd.partition_broadcast(bv_bc[:], bv_row[:])
        bout_row = pa.tile([1, D], f32)
        nc.sync.dma_start(bout_row[:], b_out[None, :])
        nc.gpsimd.partition_broadcast(bout_bc[:], bout_row[:])
        with tc.tile_pool(name="xstage", bufs=2) as xstage:
            xT = pa.tile([P, DT, S], f32r)  # 4 MB, lives through B+C
            for si in range(ST):
                x_tile = xstage.tile([P, D], f32, tag="x")
                nc.sync.dma_start(x_tile[:], X[si * P : (si + 1) * P, :])
                for dj in range(DT):
                    tp = psum.tile([P, P], f32, tag="mm")
                    nc.tensor.transpose(
                        tp[:], x_tile[:, dj * P : (dj + 1) * P], identity[:]
                    )
                    nc.vector.tensor_copy(xT[:, dj, si * P : (si + 1) * P], tp[:])

            # ---------------- phase B: Q^T | K^T projection ----------------
            with tc.tile_pool(name="wqk", bufs=2) as wqk:
                for nt in [x for p in range(PAIRS) for x in (p, PAIRS + p)]:
                    w_tile = wqk.tile([P, DT, P], f32r, tag="w")
                    nc.sync.dma_start(
                        w_tile[:],
                        w_in_kp[:, :, nt * P : (nt + 1) * P].bitcast(f32r),
                    )
                    for sc in range(SC):
                        ps = psum.tile([P, 512], f32, tag="mm")
                        for dk in range(DT):
                            nc.tensor.matmul(
                                ps[:],
                                w_tile[:, dk, :],
                                xT[:, dk, sc * 512 : (sc + 1) * 512],
                                start=(dk == 0),
                                stop=(dk == DT - 1),
                            )
                        nc.vector.tensor_scalar_add(
                            qkT[:, nt, sc * 512 : (sc + 1) * 512],
                            ps[:],
                            bqk[:, nt : nt + 1],
                        )

            # ---------------- phase C: V projection (natural) ----------------
            with tc.tile_pool(name="wv", bufs=2) as wv:
                for ncx in range(SC):
                    wv_tile = wv.tile([P, DT, 512], f32r, tag="wv")
                    nc.sync.dma_start(
                        wv_tile[:],
                        w_in_kp[
                            :, :, 2 * D + ncx * 512 : 2 * D + (ncx + 1) * 512
                        ].bitcast(f32r),
                    )
                    for st in range(ST):
                        ps = psum.tile([P, 512], f32, tag="mm")
                        for dk in range(DT):
                            nc.tensor.matmul(
                                ps[:],
                                xT[:, dk, st * P : (st + 1) * P],
                                wv_tile[:, dk, :],
                                start=(dk == 0),
                                stop=(dk == DT - 1),
                            )
                        nc.vector.tensor_tensor(
                            v_aug[:, st, ncx * 8 : (ncx + 1) * 8, 0:DK],
                            ps[:].rearrange("p (h d) -> p h d", d=DK),
                            bv_bc[:, ncx * 512 : (ncx + 1) * 512].rearrange(
                                "p (h d) -> p h d", d=DK
                            ),
                            ADD,
                        )

        pa.release()

        # ---------------- phase D: attention ----------------
        attnT_pool = tc.alloc_tile_pool(name="attnT", bufs=1)
        attnT = attnT_pool.tile([P, DT, S], f32r)  # 4 MB
        wout_pool = tc.alloc_tile_pool(name="wout", bufs=1)
        wout = wout_pool.tile([P, DT, D], f32r)  # 4 MB; prefetch during D
        nc.sync.dma_start(wout[:], w_out_kp[:].bitcast(f32r))

        with (
            tc.tile_pool(name="expp", bufs=4) as expp,
            tc.tile_pool(name="bcp", bufs=4) as bcp,
            tc.tile_pool(name="rrow", bufs=4) as rrowp,
        ):
            for pr in range(PAIRS):
                for sc in range(SC):
                    pv = [pvps.tile([P, 512], f32, tag="pv", name=f"pv{i}") for i in range(2)]
                    exps = {}
                    # software pipeline: scores(sk) on PE, exp(sk) on ACT,
                    # pv(sk-1) on PE
                    for sk in range(ST + 1):
                        if sk < ST:
                            for hh in range(2):
                                base = hh * DK
                                sps = psum.tile([P, 512], f32, tag="mm")
                                nc.tensor.matmul(
                                    sps[:],
                                    qkT[
                                        base : base + DK,
                                        PAIRS + pr,
                                        sk * P : (sk + 1) * P,
                                    ],
                                    qkT[
                                        base : base + DK,
                                        pr,
                                        sc * 512 : (sc + 1) * 512,
                                    ],
                                    start=True,
                                    stop=True,
                                )
                                ex = expp.tile([P, 512], f32r, tag="ex")
                                nc.scalar.activation(
                                    ex[:], sps[:], EXP, scale=1.0 / np.sqrt(DK)
                                )
                                exps[(sk, hh)] = ex
                        if sk >= 1:
                            for hh in range(2):
                                h = 2 * pr + hh
                                nc.tensor.matmul(
                                    pv[hh][0 : DK + 1, :],
                                    v_aug[:, sk - 1, h, :],
                                    exps.pop((sk - 1, hh))[:],
                                    start=(sk - 1 == 0),
                                    stop=(sk - 1 == ST - 1),
                                )
                    for hh in range(2):
                        base = hh * DK
                        rrow = rrowp.tile([1, 512], f32, tag="rr", name="rrow")
                        nc.vector.reciprocal(rrow[:], pv[hh][DK : DK + 1, :])
                        # full-tile broadcast (sliced variants are broken on HW)
                        bc = bcp.tile([P, 512], f32, tag="bc", name="bc")
                        nc.gpsimd.partition_broadcast(bc[:], rrow[:])
                        # attnT half = pv rows (PSUM, base 0) * bc rows (SBUF)
                        nc.vector.tensor_tensor(
                            attnT[
                                base : base + DK, pr, sc * 512 : (sc + 1) * 512
                            ],
                            pv[hh][0:DK, :],
                            bc[0:DK, :],
                            MULT,
                        )

        # ---------------- phase E: output projection ----------------
        with tc.tile_pool(name="ypool", bufs=3) as ypool:
            for st in range(ST):
                for ncx in range(SC):
                    ps = psum.tile([P, 512], f32, tag="mm")
                    for dk in range(DT):
                        nc.tensor.matmul(
                            ps[:],
                            attnT[:, dk, st * P : (st + 1) * P],
                            wout[:, dk, ncx * 512 : (ncx + 1) * 512],
                            start=(dk == 0),
                            stop=(dk == DT - 1),
                        )
                    y = ypool.tile([P, 512], f32, tag="y")
                    nc.vector.tensor_tensor(
                        y[:], ps[:], bout_bc[:, ncx * 512 : (ncx + 1) * 512], ADD
                    )
                    nc.sync.dma_start(
                        out[st * P : (st + 1) * P, ncx * 512 : (ncx + 1) * 512],
                        y[:],
                    )

        for pool in (wout_pool, attnT_pool, vaug_pool, qkT_pool, pvps, psum, const):
            pool.release()

    nc.finalize()
    return nc


_NC_CACHE = {}


def get_nc():
    if "nc" not in _NC_CACHE:
        _NC_CACHE["nc"] = build_nc()
    return _NC_CACHE["nc"]


def kernel(X, W_in, b_in, W_out, b_out):
    X = np.ascontiguousarray(np.asarray(X, dtype=np.float32))
    W_in = np.ascontiguousarray(np.asarray(W_in, dtype=np.float32))
    b_in = np.ascontiguousarray(np.asarray(b_in, dtype=np.float32))
    W_out = np.ascontiguousarray(np.asarray(W_out, dtype=np.float32))
    b_out = np.ascontiguousarray(np.asarray(b_out, dtype=np.float32))

    nc = get_nc()
    in_maps = [
        {"X": X[i], "W_in": W_in, "b_in": b_in, "W_out": W_out, "b_out": b_out}
        for i in range(B)
    ]
    res = run_bass_kernel_spmd(nc, in_maps, core_ids=list(range(B)))
    return np.stack([res.results[i]["out"] for i in range(B)], axis=0)


# revision 12
# speedup vs baseline: 1.8106x; 1.1429x over previous
"""Multi-head attention (B=8, S=1024, D=1024, H=16) on 8 TRN2 NeuronCores.

Sharding: pure data parallel — batch element b on core b. Weights are
broadcast to every core. No collectives.

Per-core algorithm (X: [S, D] for one batch element):
  1. X^T via PE transposes (fp32 has no DMA transpose).
  2. QK^T = W_in[:, :2D]^T @ X  -> [2D, S] "transposed" projection, so
     Q^T/K^T land with head dims on partitions (what scores need).
  3. V = X @ W_in[:, 2D:]       -> [S, D] natural layout, stored as
     V_aug[sk, head, 65] with a ones column appended (col 64).
  4. Per head pair (2 heads share a 128-partition group):
     S^T[sk, sq] = K_h^T.T @ Q_h^T  (K=64 contraction; the two heads run
     concurrently on PE row-groups 0-63 / 64-127),
     exp on ScalarE with scale=1/8 (softmax max-subtraction is skipped:
     scores are ~N(0,1), exp cannot overflow),
     PV: [V_h | 1]^T @ exp -> rows 0-63 unnormalized out^T, row 64 the
     softmax denominator.
  5. Normalize: reciprocal of row 64, GpSimd partition-broadcast, one
     DVE multiply per pair -> attn_out^T (f32r).
  6. Y = attn_out @ W_out + b_out in natural layout; DMA out.

All matmul operands are float32r (full-rate fp32 matmul; ~1.5e-4 rel err
measured on HW vs fp64 for K=1024).
"""

import os
import sys

sys.path.insert(0, "/opt/trn_rl_repo")

import numpy as np

import concourse.bacc as bacc
import concourse.mybir as mybir
from concourse.bass_utils import run_bass_kernel_spmd
from concourse.masks import make_identity
from concourse.tile import TileContext

B = 8
S = 1024
D = 1024
H = 16
DK = D // H  # 64
P = 128
ST = S // P   # 8 s-tiles
DT = D // P   # 8 d-tiles
NTQK = 2 * D // P  # 16 n-tiles for the Q|K part
PAIRS = H // 2     # 8 head pairs
SC = S // 512      # 2 sq chunks of 512

f32 = mybir.dt.float32
f32r = mybir.dt.float32r
bf16 = mybir.dt.bfloat16
EXP = mybir.ActivationFunctionType.Exp
MULT = mybir.AluOpType.mult
ADD = mybir.AluOpType.add


def build_nc():
    nc = bacc.Bacc()
    X = nc.dram_tensor("X", [S, D], f32, kind="ExternalInput")
    W_in = nc.dram_tensor("W_in", [D, 3 * D], f32, kind="ExternalInput")
    b_in = nc.dram_tensor("b_in", [3 * D], f32, kind="ExternalInput")
    W_out = nc.dram_tensor("W_out", [D, D], f32, kind="ExternalInput")
    b_out = nc.dram_tensor("b_out", [D], f32, kind="ExternalInput")
    out = nc.dram_tensor("out", [S, D], f32, kind="ExternalOutput")

    w_in_kp = W_in.rearrange("(ko p) n -> p ko n", p=P)  # [128, 8, 3072]
    w_out_kp = W_out.rearrange("(ko p) n -> p ko n", p=P)  # [128, 8, 1024]

    with TileContext(nc) as tc:
        const = tc.alloc_tile_pool(name="const", bufs=1)
        psum = tc.alloc_tile_pool(name="psum", bufs=4, space="PSUM")
        pvps = tc.alloc_tile_pool(name="pvps", bufs=4, space="PSUM")

        identity = const.tile([P, P], f32)
        make_identity(nc, identity[:])
        # b_in for the Q|K part, striped so bias is per-partition: [128, 16]
        bqk = const.tile([P, NTQK], f32)
        nc.sync.dma_start(bqk[:], b_in[0 : 2 * D].rearrange("(o p) -> p o", p=P))
        # b_in V part + b_out broadcast across partitions (rows staged in pa)
        bv_bc = const.tile([P, D], f32)
        bout_bc = const.tile([P, D], f32)
        ones4 = const.tile([P, ST, H, 1], f32)
        nc.vector.memset(ones4[:], 1.0)

        # ---------------- resident tensors ----------------
        qkT_pool = tc.alloc_tile_pool(name="qkT", bufs=1)
        qkT = qkT_pool.tile([P, NTQK, S], bf16)  # 4 MB
        vaug_pool = tc.alloc_tile_pool(name="vaug", bufs=1)
        v_aug = vaug_pool.tile([P, ST, H, DK + 1], bf16)  # 2.1 MB
        nc.vector.tensor_copy(v_aug[:, :, :, DK : DK + 1], ones4[:])

        # ---------------- phase A: X^T ----------------
        pa = tc.alloc_tile_pool(name="phaseA", bufs=1)
        bv_row = pa.tile([1, D], f32)
        nc.sync.dma_start(bv_row[:], b_in[None, 2 * D : 3 * D])
        nc.gpsimd.partition_broadcast(bv_bc[:], bv_row[:])
        bout_row = pa.tile([1, D], f32)
        nc.sync.dma_start(bout_row[:], b_out[None, :])
        nc.gpsimd.partition_broadcast(bout_bc[:], bout_row[:])
        with tc.tile_pool(name="xstage", bufs=2) as xstage:
            xT = pa.tile([P, DT, S], f32r)  # 4 MB, lives through B+C
            for si in range(ST):
                x_tile = xstage.tile([P, D], f32, tag="x")
                nc.sync.dma_start(x_tile[:], X[si * P : (si + 1) * P, :])
                for dj in range(DT):
                    tp = psum.tile([P, P], f32, tag="mm")
                    nc.tensor.transpose(
                        tp[:], x_tile[:, dj * P : (dj + 1) * P], identity[:]
                    )
                    nc.vector.tensor_copy(xT[:, dj, si * P : (si + 1) * P], tp[:])

            # ---------------- phase B: Q^T | K^T projection ----------------
            with tc.tile_pool(name="wqk", bufs=2) as wqk:
                for nt in [x for p in range(PAIRS) for x in (p, PAIRS + p)]:
                    w_tile = wqk.tile([P, DT, P], f32r, tag="w")
                    nc.sync.dma_start(
                        w_tile[:],
                        w_in_kp[:, :, nt * P : (nt + 1) * P].bitcast(f32r),
                    )
                    for sc in range(SC):
                        ps = psum.tile([P, 512], f32, tag="mm")
                        for dk in range(DT):
                            nc.tensor.matmul(
                                ps[:],
                                w_tile[:, dk, :],
                                xT[:, dk, sc * 512 : (sc + 1) * 512],
                                start=(dk == 0),
                                stop=(dk == DT - 1),
                            )
                        nc.vector.tensor_scalar_add(
                            qkT[:, nt, sc * 512 : (sc + 1) * 512],
                            ps[:],
                            bqk[:, nt : nt + 1],
                        )

            # ---------------- phase C: V projection (natural) ----------------
            with tc.tile_pool(name="wv", bufs=2) as wv:
                for ncx in range(SC):
                    wv_tile = wv.tile([P, DT, 512], f32r, tag="wv")
                    nc.sync.dma_start(
                        wv_tile[:],
                        w_in_kp[
                            :, :, 2 * D + ncx * 512 : 2 * D + (ncx + 1) * 512
                        ].bitcast(f32r),
                    )
                    for st in range(ST):
                        ps = psum.tile([P, 512], f32, tag="mm")
                        for dk in range(DT):
                            nc.tensor.matmul(
                                ps[:],
                                xT[:, dk, st * P : (st + 1) * P],
                                wv_tile[:, dk, :],
                                start=(dk == 0),
                                stop=(dk == DT - 1),
                            )
                        nc.vector.tensor_tensor(
                            v_aug[:, st, ncx * 8 : (ncx + 1) * 8, 0:DK],
                            ps[:].rearrange("p (h d) -> p h d", d=DK),
                            bv_bc[:, ncx * 512 : (ncx + 1) * 512].rearrange(
                                "p (h d) -> p h d", d=DK
                            ),
                            ADD,
                        )

        pa.release()

        # ---------------- phase D: attention ----------------
        attnT_pool = tc.alloc_tile_pool(name="attnT", bufs=1)
        attnT = attnT_pool.tile([P, DT, S], f32r)  # 4 MB
        wout_pool = tc.alloc_tile_pool(name="wout", bufs=1)
        wout = wout_pool.tile([P, DT, D], f32r)  # 4 MB; prefetch during D
        nc.sync.dma_start(wout[:], w_out_kp[:].bitcast(f32r))

        with (
            tc.tile_pool(name="expp", bufs=4) as expp,
            tc.tile_pool(name="bcp", bufs=4) as bcp,
            tc.tile_pool(name="rrow", bufs=4) as rrowp,
        ):
            for pr in range(PAIRS):
                for sc in range(SC):
                    pv = [pvps.tile([P, 512], f32, tag="pv", name=f"pv{i}") for i in range(2)]
                    exps = {}
                    # software pipeline: scores(sk) on PE, exp(sk) on ACT,
                    # pv(sk-1) on PE
                    for sk in range(ST + 1):
                        if sk < ST:
                            for hh in range(2):
                                base = hh * DK
                                sps = psum.tile([P, 512], f32, tag="mm")
                                nc.tensor.matmul(
                                    sps[:],
                                    qkT[
                                        base : base + DK,
                                        PAIRS + pr,
                                        sk * P : (sk + 1) * P,
                                    ],
                                    qkT[
                                        base : base + DK,
                                        pr,
                                        sc * 512 : (sc + 1) * 512,
                                    ],
                                    start=True,
                                    stop=True,
                                )
                                ex = expp.tile([P, 512], bf16, tag="ex")
                                nc.scalar.activation(
                                    ex[:], sps[:], EXP, scale=1.0 / np.sqrt(DK)
                                )
                                exps[(sk, hh)] = ex
                        if sk >= 1:
                            for hh in range(2):
                                h = 2 * pr + hh
                                nc.tensor.matmul(
                                    pv[hh][0 : DK + 1, :],
                                    v_aug[:, sk - 1, h, :],
                                    exps.pop((sk - 1, hh))[:],
                                    start=(sk - 1 == 0),
                                    stop=(sk - 1 == ST - 1),
                                )
                    for hh in range(2):
                        base = hh * DK
                        rrow = rrowp.tile([1, 512], f32, tag="rr", name="rrow")
                        nc.vector.reciprocal(rrow[:], pv[hh][DK : DK + 1, :])
                        # full-tile broadcast (sliced variants are broken on HW)
                        bc = bcp.tile([P, 512], f32, tag="bc", name="bc")
                        nc.gpsimd.partition_broadcast(bc[:], rrow[:])
                        # attnT half = pv rows (PSUM, base 0) * bc rows (SBUF)
                        nc.vector.tensor_tensor(
                            attnT[
                                base : base + DK, pr, sc * 512 : (sc + 1) * 512
                            ],
                            pv[hh][0:DK, :],
                            bc[0:DK, :],
                            MULT,
                        )

        # ---------------- phase E: output projection ----------------
        with tc.tile_pool(name="ypool", bufs=3) as ypool:
            for st in range(ST):
                for ncx in range(SC):
                    ps = psum.tile([P, 512], f32, tag="mm")
                    for dk in range(DT):
                        nc.tensor.matmul(
                            ps[:],
                            attnT[:, dk, st * P : (st + 1) * P],
                            wout[:, dk, ncx * 512 : (ncx + 1) * 512],
                            start=(dk == 0),
                            stop=(dk == DT - 1),
                        )
                    y = ypool.tile([P, 512], f32, tag="y")
                    nc.vector.tensor_tensor(
                        y[:], ps[:], bout_bc[:, ncx * 512 : (ncx + 1) * 512], ADD
                    )
                    nc.sync.dma_start(
                        out[st * P : (st + 1) * P, ncx * 512 : (ncx + 1) * 512],
                        y[:],
                    )

        for pool in (wout_pool, attnT_pool, vaug_pool, qkT_pool, pvps, psum, const):
            pool.release()

    nc.finalize()
    return nc


_NC_CACHE = {}


def get_nc():
    if "nc" not in _NC_CACHE:
        _NC_CACHE["nc"] = build_nc()
    return _NC_CACHE["nc"]


def kernel(X, W_in, b_in, W_out, b_out):
    X = np.ascontiguousarray(np.asarray(X, dtype=np.float32))
    W_in = np.ascontiguousarray(np.asarray(W_in, dtype=np.float32))
    b_in = np.ascontiguousarray(np.asarray(b_in, dtype=np.float32))
    W_out = np.ascontiguousarray(np.asarray(W_out, dtype=np.float32))
    b_out = np.ascontiguousarray(np.asarray(b_out, dtype=np.float32))

    nc = get_nc()
    in_maps = [
        {"X": X[i], "W_in": W_in, "b_in": b_in, "W_out": W_out, "b_out": b_out}
        for i in range(B)
    ]
    res = run_bass_kernel_spmd(nc, in_maps, core_ids=list(range(B)))
    return np.stack([res.results[i]["out"] for i in range(B)], axis=0)
